# revision 18
# baseline (speedup 1.0000x reference)
"""Trainium2 Bass kernel for CustomGConvLSTM (single step from zero state).

Math (verified against reference): with H=C=0 the hidden-path ChebConvs reduce
to their biases and the forget gate is multiplied by C=0, so per period p:

    Y_p = A @ X_p            (A = -D^-1/2 W D^-1/2, scatter over E edges)
    Z   = [X_p; Y_p] @ [W0; W1] + b          (gates i, c, o)
    I = sig(Z_i); T = tanh(Z_c); Cn = I*T
    O = sig(Z_o + wc[2]*Cn);  out += O * tanh(Cn)

Distribution: edges sharded by dst range across 8 cores (6250 nodes/core);
periods all handled together (gather rows carry all 8 periods).

Per core, per 128-dst-node window:
  - Per-chunk indirect DMAs gather the window's edge source rows
    (bf16, 1KB each, all periods at once) from a replicated table in DRAM.
    (HW indirect DMA takes one index per partition per instruction, so the
    gather is 128 rows per call; this SWDGE descriptor-generation serializes
    on GpSimd and is the kernel's critical path.)
  - One-hot scatter matrices built on DVE with a single fused
    tensor_scalar(is_equal, mult) per 128-edge chunk.
  - PE matmul accumulates the window's Y [128 nodes, 8*64] in PSUM.
  - PE transposes flip Y to feature-major into partitions 64..127 of an
    [X^T; Y^T] tile (X^T DMA'd into partitions 0..63).
  - Dense gates: one 128-contraction matmul per gate; peephole wc2*Cn and
    biases folded into PSUM/activation; pointwise in bf16.
"""
import os
import numpy as np
import ml_dtypes

import concourse.bass as bass
import concourse.mybir as mybir
import concourse.tile as tile
from concourse.bass import IndirectOffsetOnAxis
from concourse.bass_utils import run_bass_kernel_spmd
from concourse.masks import make_identity

F32 = mybir.dt.float32
BF16 = mybir.dt.bfloat16
I32 = mybir.dt.int32
AF = mybir.ActivationFunctionType
OP = mybir.AluOpType
BF16NP = ml_dtypes.bfloat16

N, FIN, FOUT, P, NCORES = 50000, 64, 64, 8, 8
NPC = N // NCORES          # 6250 nodes per core
WSZ = 128                  # window (node slot) size
NWIN = (NPC + WSZ - 1) // WSZ      # 49 windows per core
NTC = NWIN * WSZ           # 6272 padded node slots per core
RL = FIN * P               # 512 (gather row: all periods, period-major)
GATES = (0, 2, 3)          # i, c, o (f is dead: multiplied by C=0)

LAST_RESULTS = None        # BassKernelResults of the last run (for test.py)
LAST_EXEC_S = None         # wall time of the device submit+run (upper bound)

_ctr = [0]


def split_multiwaits(nc):
    """Split multi-wait instructions into single-wait NoOps.

    This environment's walrus codegen rejects instructions carrying more
    than one sync wait ("Too many sync wait commands"). Hoist all but one
    wait onto NoOps placed immediately before, on the same engine.
    """
    n_split = 0
    for f in nc.m.functions:
        for bb in f.blocks:
            insts = bb.instructions
            if not any(
                i.sync_info is not None and i.sync_info.on_wait
                and len(i.sync_info.on_wait) > 1
                for i in insts
            ):
                continue
            new = []
            for inst in insts:
                si = inst.sync_info
                if si is not None and si.on_wait and len(si.on_wait) > 1:
                    waits = list(si.on_wait)
                    for wcond in waits[:-1]:
                        _ctr[0] += 1
                        nop = mybir.InstNoOp(
                            name=f"I-waitsplit-{_ctr[0]}", ins=[], outs=[])
                        nop.engine = inst.engine
                        nop.sync_info = mybir.SyncInfo(
                            on_wait=[wcond], on_update=[])
                        new.append(nop)
                    inst.sync_info = mybir.SyncInfo(
                        on_wait=[waits[-1]],
                        on_update=list(si.on_update or []))
                    n_split += 1
                new.append(inst)
            bb.instructions = new
    return n_split


def _groups():
    """Window groups for the dense phase: 12 groups of 4 + 1 group of 1."""
    gs = [list(range(i, i + 4)) for i in range(0, 48, 4)]
    gs.append([48])
    return gs


def _build_program(counts):
    off = np.concatenate([[0], np.cumsum(counts)]).astype(int)
    nch = int(off[-1])
    nc = bass.Bass()
    xg_d = nc.dram_tensor("xg", [N, RL], BF16, kind="ExternalInput")
    xt_d = nc.dram_tensor("xt", [FIN, P, NTC], BF16, kind="ExternalInput")
    srcs_d = nc.dram_tensor("srcs", [128, nch], I32, kind="ExternalInput")
    slots_d = nc.dram_tensor("slots", [128, nch], F32, kind="ExternalInput")
    wns_d = nc.dram_tensor("wns", [128, nch], F32, kind="ExternalInput")
    w01_d = nc.dram_tensor("w01", [128, 192], BF16, kind="ExternalInput")
    wc2d_d = nc.dram_tensor("wc2d", [FOUT, FOUT], BF16, kind="ExternalInput")
    bias_d = nc.dram_tensor("bias", [FOUT, 3], F32, kind="ExternalInput")
    out_d = nc.dram_tensor("out", [FOUT, NTC], F32, kind="ExternalOutput")

    with tile.TileContext(nc) as tc:
        with tc.tile_pool(name="const", bufs=1) as cp:
            srcs_sb = cp.tile([128, nch], I32)
            nc.sync.dma_start(srcs_sb[:], srcs_d[:])
            slots_sb = cp.tile([128, nch], F32)
            nc.sync.dma_start(slots_sb[:], slots_d[:])
            wns_sb = cp.tile([128, nch], F32)
            nc.sync.dma_start(wns_sb[:], wns_d[:])
            iota_i = cp.tile([128, 128], I32)
            nc.gpsimd.iota(iota_i[:], pattern=[[1, 128]], base=0,
                           channel_multiplier=0)
            iota_b = cp.tile([128, 128], BF16)
            nc.vector.tensor_copy(iota_b[:], iota_i[:])
            ident_b = cp.tile([128, 128], BF16)
            make_identity(nc, ident_b[:])
            w01_sb = cp.tile([128, 192], BF16)
            nc.sync.dma_start(w01_sb[:], w01_d[:])
            wc2d_sb = cp.tile([FOUT, FOUT], BF16)
            nc.sync.dma_start(wc2d_sb[:], wc2d_d[:])
            bias_sb = cp.tile([FOUT, 3], F32)
            nc.sync.dma_start(bias_sb[:], bias_d[:])
            acc = cp.tile([FOUT, NTC], F32)

            with (
                tc.tile_pool(name="gather", bufs=3) as gp,
                tc.tile_pool(name="sgen", bufs=4) as sp,
                tc.tile_pool(name="yw", bufs=2) as yp,
                tc.tile_pool(name="xy", bufs=6) as xyp,
                tc.tile_pool(name="acts", bufs=2) as ap_,
                tc.tile_pool(name="py", bufs=1, space="PSUM") as pyp,
                tc.tile_pool(name="pt", bufs=1, space="PSUM") as ptp,
                tc.tile_pool(name="pz", bufs=2, space="PSUM") as pzp,
            ):
                kmax = int(max(counts))

                def scatter_window(w):
                    k = int(counts[w])
                    g_tile = gp.tile([128, kmax * RL], BF16, tag="g")
                    # HW indirect DMA consumes ONE index per partition per
                    # instruction (verified: extra offset-AP columns read
                    # consecutive rows instead) — issue one per chunk.
                    for c in range(k):
                        nc.gpsimd.indirect_dma_start(
                            out=g_tile[:, c * RL:(c + 1) * RL],
                            out_offset=None,
                            in_=xg_d[:],
                            in_offset=IndirectOffsetOnAxis(
                                ap=srcs_sb[:, int(off[w]) + c:
                                           int(off[w]) + c + 1],
                                axis=0),
                        )
                    py = pyp.tile([128, RL], F32)
                    for c in range(k):
                        ch = int(off[w]) + c
                        s_t = sp.tile([128, 128], BF16, tag="s")
                        nc.vector.tensor_scalar(
                            out=s_t[:], in0=iota_b[:],
                            scalar1=slots_sb[:, ch:ch + 1],
                            scalar2=wns_sb[:, ch:ch + 1],
                            op0=OP.is_equal, op1=OP.mult)
                        nc.tensor.matmul(
                            py[:], lhsT=s_t[:],
                            rhs=g_tile[:, c * RL:(c + 1) * RL],
                            start=(c == 0), stop=(c == k - 1))
                    ywb = yp.tile([128, RL], BF16, tag="ywb")
                    nc.vector.tensor_copy(ywb[:], py[:])
                    pt = ptp.tile([128, P, 128], BF16)
                    for p_ in range(P):
                        nc.tensor.transpose(
                            pt[64:128, p_, :],
                            ywb[:, p_ * FOUT:(p_ + 1) * FOUT],
                            ident_b[:])
                    xy = xyp.tile([128, P, 128], BF16, tag="xy")
                    nc.sync.dma_start(
                        xy[0:64, :, :],
                        xt_d[:, :, w * 128:(w + 1) * 128])
                    nc.vector.tensor_copy(xy[64:128, :, :], pt[64:128, :, :])
                    return xy

                def body():
                    nc.vector.memset(acc[:], 0.0)
                    for wlist in _groups():
                        run_group(wlist)
                    nc.sync.dma_start(out_d[:], acc[:])

                def run_group(wlist):
                    xys = [scatter_window(w) for w in wlist]
                    nw = len(wlist)
                    ncols = nw * 128
                    gbase = wlist[0] * 128
                    for p_ in range(P):
                        z = pzp.tile([FOUT, 1536], F32, tag="z")
                        for wi, xy in enumerate(xys):
                            for gi in range(3):
                                # one accumulation group per gate bank:
                                # windows write disjoint columns inside it
                                nc.tensor.matmul(
                                    z[:, gi * 512 + wi * 128:
                                      gi * 512 + wi * 128 + 128],
                                    lhsT=w01_sb[:, gi * 64:(gi + 1) * 64],
                                    rhs=xy[:, p_, :],
                                    start=(wi == 0),
                                    stop=(gi != 2 and wi == nw - 1))
                        I_t = ap_.tile([FOUT, 512], BF16, tag="I")
                        nc.scalar.activation(
                            I_t[:, :ncols], z[:, 0:ncols], AF.Sigmoid,
                            bias=bias_sb[:, 0:1])
                        T_t = ap_.tile([FOUT, 512], BF16, tag="T")
                        nc.scalar.activation(
                            T_t[:, :ncols], z[:, 512:512 + ncols], AF.Tanh,
                            bias=bias_sb[:, 1:2])
                        C_t = ap_.tile([FOUT, 512], BF16, tag="C")
                        nc.vector.tensor_tensor(
                            out=C_t[:, :ncols], in0=I_t[:, :ncols],
                            in1=T_t[:, :ncols], op=OP.mult)
                        nc.tensor.matmul(
                            z[:, 1024:1024 + ncols], lhsT=wc2d_sb[:],
                            rhs=C_t[:, :ncols], start=False, stop=True)
                        th_t = ap_.tile([FOUT, 512], BF16, tag="th")
                        nc.scalar.activation(
                            th_t[:, :ncols], C_t[:, :ncols], AF.Tanh)
                        O_t = ap_.tile([FOUT, 512], BF16, tag="O")
                        nc.scalar.activation(
                            O_t[:, :ncols], z[:, 1024:1024 + ncols],
                            AF.Sigmoid, bias=bias_sb[:, 2:3])
                        pr_t = ap_.tile([FOUT, 512], BF16, tag="pr")
                        nc.vector.tensor_tensor(
                            out=pr_t[:, :ncols], in0=O_t[:, :ncols],
                            in1=th_t[:, :ncols], op=OP.mult)
                        nc.vector.tensor_tensor(
                            out=acc[:, gbase:gbase + ncols],
                            in0=acc[:, gbase:gbase + ncols],
                            in1=pr_t[:, :ncols], op=OP.add)

                body()
    return nc


def _prepare(X, edge_index, edge_weight):
    """Host-side edge normalization, sorting, sharding and table building."""
    X = np.asarray(X, dtype=np.float32)
    ei = np.asarray(edge_index).astype(np.int64)
    ew64 = np.asarray(edge_weight).astype(np.float64)
    src, dst = ei[0], ei[1]

    deg = np.bincount(src, weights=ew64, minlength=N)
    dis = np.where(deg > 0, 1.0 / np.sqrt(np.maximum(deg, 1e-12)), 0.0)
    wn = (-dis[src] * ew64 * dis[dst]).astype(np.float32)

    # Degree-aware window packing, per core: the chunk count per window is
    # ceil(edges/128) maxed over cores, and windows average 2049 edges —
    # just above the 16-chunk boundary. Put the ~106 heaviest-in-degree
    # nodes in the short last window and LPT-balance the other 6144 nodes
    # into 48 windows of exactly 128 nodes, keeping every window <= 2048
    # edges on every core: 48*16 + ~23 = ~791 gather calls instead of 830.
    NHEAVY = NPC - (NWIN - 1) * WSZ      # 106 nodes in the last window
    perms = np.full((NCORES, NTC), -1, np.int64)   # slot -> global node id
    win_of = np.empty((NCORES, NPC), np.int32)
    slot_of = np.empty((NCORES, NPC), np.int32)
    for c in range(NCORES):
        base = c * NPC
        mask = (dst >= base) & (dst < base + NPC)
        d = np.bincount(dst[mask] - base, minlength=NPC)
        order_d = np.argsort(-d, kind="stable")
        heavy, rest = order_d[:NHEAVY], order_d[NHEAVY:]
        sums = np.zeros(NWIN - 1, np.int64)
        cnts_b = np.zeros(NWIN - 1, np.int64)
        for n in rest:                      # desc by degree: balanced LPT
            elig = np.where(cnts_b < WSZ)[0]
            b = int(elig[np.argmin(sums[elig])])
            win_of[c, n] = b
            slot_of[c, n] = cnts_b[b]
            perms[c, b * WSZ + cnts_b[b]] = base + n
            sums[b] += d[n]
            cnts_b[b] += 1
        for s, n in enumerate(heavy):
            win_of[c, n] = NWIN - 1
            slot_of[c, n] = s
            perms[c, (NWIN - 1) * WSZ + s] = base + n

    # per (core, window) edge lists in packed order
    core_of = dst // NPC
    ew_w = np.empty((NCORES, NWIN), object)
    cnts = np.zeros((NCORES, NWIN), np.int64)
    for c in range(NCORES):
        emask = core_of == c
        e_idx = np.nonzero(emask)[0]
        w_e = win_of[c, dst[e_idx] - c * NPC]
        order_e = np.argsort(w_e, kind="stable")
        e_sorted = e_idx[order_e]
        w_sorted = w_e[order_e]
        bounds = np.searchsorted(w_sorted, np.arange(NWIN + 1))
        for w in range(NWIN):
            ew_w[c, w] = e_sorted[bounds[w]:bounds[w + 1]]
            cnts[c, w] = bounds[w + 1] - bounds[w]

    counts = np.maximum(1, -(-cnts // 128)).max(axis=0)  # [NWIN] chunk counts
    off = np.concatenate([[0], np.cumsum(counts)]).astype(int)
    nch = int(off[-1])

    srcs_all = np.zeros((NCORES, nch * 128), np.int32)
    slots_all = np.zeros((NCORES, nch * 128), np.float32)
    wns_all = np.zeros((NCORES, nch * 128), np.float32)
    for c in range(NCORES):
        for w in range(NWIN):
            es = ew_w[c, w]
            cnt = len(es)
            o = int(off[w]) * 128
            srcs_all[c, o:o + cnt] = src[es]
            slots_all[c, o:o + cnt] = slot_of[c, dst[es] - c * NPC].astype(
                np.float32)
            wns_all[c, o:o + cnt] = wn[es]

    Xg = np.ascontiguousarray(
        X.transpose(0, 2, 1)).reshape(N, RL).astype(BF16NP)
    Xt = np.ascontiguousarray(X.transpose(1, 2, 0))  # [64, 8, N]
    return counts, off, nch, srcs_all, slots_all, wns_all, Xg, Xt, perms


def _weights_tables(Wx0, Wx1, bx, bh, bg, wc):
    w01 = np.zeros((128, 192), np.float32)
    biases = np.zeros((FOUT, 3), np.float32)
    for gi, g in enumerate(GATES):
        w01[:FIN, gi * 64:(gi + 1) * 64] = np.asarray(Wx0[g], np.float32)
        w01[FIN:, gi * 64:(gi + 1) * 64] = np.asarray(Wx1[g], np.float32)
        biases[:, gi] = np.asarray(bx[g] + bh[g] + bg[g], np.float32)
    wc2d = np.diag(np.asarray(wc[2], np.float32)).astype(BF16NP)
    return w01.astype(BF16NP), wc2d, biases


def build_all(X, edge_index, edge_weight, Wx0, Wx1, bx, Wh0, Wh1, bh, wc, bg,
              split=True):
    """Build (nc, in_maps) — shared by the device path and sim harnesses."""
    counts, off, nch, srcs_all, slots_all, wns_all, Xg, Xt, perms = _prepare(
        X, edge_index, edge_weight)
    w01, wc2d, biases = _weights_tables(Wx0, Wx1, bx, bh, bg, wc)

    in_maps = []
    for c in range(NCORES):
        valid = perms[c] >= 0
        xt_c = np.zeros((FIN, P, NTC), BF16NP)
        xt_c[:, :, valid] = Xt[:, :, perms[c][valid]].astype(BF16NP)
        in_maps.append(dict(
            xg=Xg,
            xt=xt_c,
            srcs=np.ascontiguousarray(srcs_all[c].reshape(nch, 128).T),
            slots=np.ascontiguousarray(slots_all[c].reshape(nch, 128).T),
            wns=np.ascontiguousarray(wns_all[c].reshape(nch, 128).T),
            w01=w01, wc2d=wc2d, bias=biases,
        ))

    nc = _build_program(counts)
    if split:
        split_multiwaits(nc)
    return nc, in_maps, perms


def kernel(X, edge_index, edge_weight, Wx0, Wx1, bx, Wh0, Wh1, bh, wc, bg):
    global LAST_RESULTS, LAST_EXEC_S
    nc, in_maps, perms = build_all(X, edge_index, edge_weight, Wx0, Wx1, bx,
                                   Wh0, Wh1, bh, wc, bg)
    trace = bool(int(os.environ.get("KERNEL_TRACE", "0")))
    import time as _time
    _t0 = _time.time()
    res = run_bass_kernel_spmd(nc, in_maps, core_ids=list(range(NCORES)),
                               trace=trace)
    LAST_EXEC_S = _time.time() - _t0
    LAST_RESULTS = res

    out = np.empty((N, FOUT), np.float32)
    for c in range(NCORES):
        valid = perms[c] >= 0
        out[perms[c][valid]] = res.results[c]["out"][:, valid].T
    return out
